# revision 16
# baseline (speedup 1.0000x reference)
"""DOSAConLoss Trainium2 kernel (v2).

result = mean(base) * (1 + ALPHA * (N/1024) / max_hist)
since sum(hist) == N exactly (every box center lands in one bin).

8-way data parallel over N. Per core:
  - per-partition partial sums of base  (acc_out [128, n_tiles])
  - packed 32x32 histogram of target box centers (hist_out [128, 32]:
    8 slot-blocks of 16 partitions; row m of a block packs y-bins
    2m / 2m+1 at radix 512)

v2 changes vs v1:
  - fp16 channel-planar inputs ([4, NB] per tensor): halves HBM/transfer
    bytes and gives contiguous step-1 operand reads (DVE 2x/4x modes).
  - all-bf16 DVE elementwise chain (DVE computes fp32 internally; only
    each op's output rounds to bf16). ln/exp reciprocals stay f32 on ACT.
  - atan difference via identity atan(r2)-atan(r1) =
    atan((w2*h1-w1*h2)/(h1*h2+w1*w2)), range-reduced to [0,1] for table
    accuracy; the sign is irrelevant because v squares the difference.
  - histogram matmuls grouped: 8 box-columns share one [128,128]
    stationary (8 x 16 packed-y one-hots, block slot s = column group
    16s..16s+15) against a [128,256] moving (8 x 32 x-one-hots). Only
    the 8 diagonal [16,32] blocks of the [128,256] psum are real counts
    (off-diagonal cross-blocks accumulate garbage, never read).
    8x fewer TensorE instructions than v1.
  - single psum accumulation chain per core (per-slot cell counts stay
    far below the radix-512 decode bound on this data).
  - no host-side tie fixup: magic-number binning differs from floor on
    ~1e-6 of boxes; the induced max_h error is a few counts (~1e-4
    relative on the result), far inside the 2e-2 gate.

Math rewrite (validated vs reference in fp64/f32/bf16 simulation):
  W=w1+w2, mx=max(|2dx|,|dW|) -> iw4=W-mx=2*iw; inter4=4*inter
  u4 = asum - inter4/4 = union - eps ; iou = inter4 / (4*u4+4*eps)
  cw2=W+mx=2cw ; c24=cw2^2+ch2^2=4c2 ; rho4=(2dx)^2+(2dy)^2 ; rho2/c2==rho4/c24
  v = (2/pi * (atan(w2/h2)-atan(w1/h1)))^2 via the atan-difference identity
  ciou = iou - rho4/c24 - v^2/(v-iou+1+eps) ; base=(1-ciou)^3/(w2*h2+1e-7)
Reciprocals via exp(-ln(x)) on ACT (ACT Reciprocal is disallowed in bass).
"""

import numpy as np

import concourse.bass as bass
import concourse.bacc as bacc
import concourse.mybir as mybir
import concourse.tile as tile
from concourse import bass_utils

# The act-table-load chooser picks the first set containing each function,
# which puts Ln in `natural_log` and Exp in `exp_and_others`, forcing a
# ~2.7us table switch at every Ln->Exp pair (we use exp(-ln(x)) for all
# reciprocals). Hide Ln/Exp from the single-function sets so the chooser
# lands on `natural_log_exp_and_others`.
_orig_get_act_tables = bacc.get_activation_tables


def _patched_get_act_tables(arch):
    t = {k: set(v) for k, v in _orig_get_act_tables(arch).items()}
    t.get("natural_log", set()).discard(mybir.ActivationFunctionType.Ln)
    t.get("exp_and_others", set()).discard(mybir.ActivationFunctionType.Exp)
    t.get("exp_and_friends", set()).discard(mybir.ActivationFunctionType.Exp)
    return t


bacc.get_activation_tables = _patched_get_act_tables

F32 = mybir.dt.float32
F16 = mybir.dt.float16
BF16 = mybir.dt.bfloat16
AF = mybir.ActivationFunctionType
OP = mybir.AluOpType

GRID = 32
ALPHA = 1.5
EPS = 1e-7
PI = float(np.pi)
MAGIC = float(2 ** 23)
# floor offset: round(s*x + CF) - 1 == floor(s*x) EXACTLY for every fp16
# x in [0,1) and s in {16, 32}: s*x sits on a power-of-2 grid no finer
# than 2^-11 relative to its magnitude, so s*x + CF stays strictly inside
# (k+0.5, k+1.5) with margin >= 2^-12 - 2^-19 (f32 add rounding). No RNE
# ties, no misbins, and py = floor(32y) - 2*floor(16y) is always in {0,1}.
CF = 0.5 + 2.0 ** -12

N_CORES = 8
N_TOTAL = 4_000_000
T_MAIN = 1024
TC_MAIN = 512
NT_MAIN = 4
NB_CORE = 128 * T_MAIN * NT_MAIN      # 524288 padded boxes per core
# pred==targ -> base contribution 0. y = 17/32 exactly -> odd bin gy=17,
# so the ~3k pad counts per (core,slot) land on the radix-512 (n1) digit
# whose bound is 32767, not on n0 whose decode bound is 511.
PAD_BOX = (0.5, 0.53125, 1.0, 1.0)    # bin (gy, gx) = (17, 16)

# GPSIMD (pool) offload: 2-src add/sub/mult ops only (tuned via profile)
GPS_OPS = {"asum", "c24", "rho4", "phh", "pww", "dent", "term2", "s12"}


def build_nc(NB, T=T_MAIN, Tc=TC_MAIN, gps=True):
    """Build the per-core Bass program. NB must equal n_tiles*128*T."""
    n_tiles = NB // (128 * T)
    assert NB == n_tiles * 128 * T
    n_chunks = T // Tc
    assert T == n_chunks * Tc
    assert Tc % 8 == 0
    n_grp8 = Tc // 8  # 8-column matmul groups per chunk

    nc = bacc.Bacc("TRN2", target_bir_lowering=False, debug=False)
    pred_d = nc.dram_tensor("pred_boxes", [4, NB], F16, kind="ExternalInput")
    targ_d = nc.dram_tensor("target_boxes", [4, NB], F16, kind="ExternalInput")
    acc_d = nc.dram_tensor("acc_out", [128, n_tiles], F32, kind="ExternalOutput")
    hist_d = nc.dram_tensor("hist_out", [128, 8 * GRID], F32, kind="ExternalOutput")

    pred_v = pred_d.ap().rearrange("c (n p t) -> n p c t", p=128, t=T)
    targ_v = targ_d.ap().rearrange("c (n p t) -> n p c t", p=128, t=T)

    def eng(name):
        return nc.gpsimd if (gps and name in GPS_OPS) else nc.vector

    with tile.TileContext(nc) as tc:
        with (
            tc.tile_pool(name="inp", bufs=2) as inp,
            tc.tile_pool(name="tmp", bufs=2) as tmp,
            tc.tile_pool(name="tmpf", bufs=1) as tmpf,
            tc.tile_pool(name="ohp", bufs=2) as ohp,
            tc.tile_pool(name="cst", bufs=1) as cst,
            tc.tile_pool(name="psp", bufs=1, space="PSUM") as psp,
        ):
            bias_tiles = {}

            def bias_ap(val):
                if val not in bias_tiles:
                    t = cst.tile([128, 1], F32, name=f"bias{len(bias_tiles)}")
                    nc.vector.memset(t[:], val)
                    bias_tiles[val] = t[:]
                return bias_tiles[val]

            acc_sb = cst.tile([128, n_tiles], F32)
            hist_sb = cst.tile([128, 8 * GRID], F32)
            ps = psp.tile([128, 8 * GRID], F32, name="ps")

            mm_total = (NB // 128) // 8
            mm_i = 0

            # Rotating bf16 temp slots (bufs=2 -> reuse distance 2*NGEN
            # allocations; max live-span below is ~9). Long-lived values
            # get dedicated tags.
            NGEN = 8
            DEDICATED = {"a2t", "iou", "term1", "vv", "rho4", "nfx", "hyb", "wyb"}
            gen_counter = [0]
            NGENF = 2
            genf_counter = [0]

            for n in range(n_tiles):
                pt = inp.tile([128, 4 * T], F16, tag="pred")
                tt = inp.tile([128, 4 * T], F16, tag="targ")
                p3 = pt.rearrange("p (c t) -> p c t", c=4)
                t3 = tt.rearrange("p (c t) -> p c t", c=4)
                nc.sync.dma_start(p3[:], pred_v[n])
                nc.sync.dma_start(t3[:], targ_v[n])
                x1, y1, w1, h1 = p3[:, 0], p3[:, 1], p3[:, 2], p3[:, 3]
                x2, y2, w2, h2 = t3[:, 0], t3[:, 1], t3[:, 2], t3[:, 3]

                def t_(tag):
                    if tag in DEDICATED:
                        return tmp.tile([128, T], BF16, tag=tag, name=tag)[:]
                    i = gen_counter[0] % NGEN
                    gen_counter[0] += 1
                    return tmp.tile([128, T], BF16, tag=f"g{i}", name=tag)[:]

                def tf_(tag):
                    i = genf_counter[0] % NGENF
                    genf_counter[0] += 1
                    return tmpf.tile([128, T], F32, tag=f"f{i}", name=tag)[:]

                # ---- histogram prep first (primes TensorE early) ----
                zmx, zmy, q1y = tf_("zmx"), tf_("zmy"), tf_("q1y")
                nfx = t_("nfx")
                nfy = t_("nfy")
                hyb = t_("hyb")
                pyb = t_("pyb")
                wyb = t_("wyb")
                nc.vector.tensor_scalar(zmx, x2, 32.0, CF, OP.mult, OP.add)
                nc.vector.tensor_scalar(nfx, zmx, MAGIC, MAGIC + 1.0, OP.add, OP.subtract)
                nc.vector.tensor_scalar(zmy, y2, 32.0, CF, OP.mult, OP.add)
                nc.vector.tensor_scalar(nfy, zmy, MAGIC, MAGIC + 1.0, OP.add, OP.subtract)
                nc.vector.tensor_scalar(q1y, y2, 16.0, CF, OP.mult, OP.add)
                nc.vector.tensor_scalar(hyb, q1y, MAGIC, MAGIC + 1.0, OP.add, OP.subtract)
                nc.vector.scalar_tensor_tensor(pyb, hyb, -2.0, nfy, OP.mult, OP.add)
                nc.vector.tensor_scalar(wyb, pyb, 511.0, 1.0, OP.mult, OP.add)

                def emit_chunk(c):
                    nonlocal mm_i
                    # ohx: bin-major [i, t] (contiguous-t writes keep 4x mode)
                    # ohy: slab-major addr(t,i) = (t//8)*128 + 8*i + t%8, so
                    # each 8-column group's stationary is ONE contiguous
                    # [128,128] slice (matmul weights allow only 1 free dim).
                    ohx = ohp.tile([128, GRID * Tc], BF16, tag="ohx", name="ohx")
                    ohy = ohp.tile([128, (GRID // 2) * Tc], BF16, tag="ohy", name="ohy")
                    s = slice(c * Tc, (c + 1) * Tc)
                    for i in range(GRID):
                        nc.vector.tensor_scalar(
                            ohx[:, i * Tc : (i + 1) * Tc], nfx[:, s],
                            float(i), None, OP.is_equal,
                        )
                    hyb3 = hyb[:, s].rearrange("p (a b) -> p a b", b=8)
                    wyb3 = wyb[:, s].rearrange("p (a b) -> p a b", b=8)
                    ohy_w = ohy.rearrange("p (a i b) -> p i a b", i=16, b=8)
                    for m in range(GRID // 2):
                        nc.vector.scalar_tensor_tensor(
                            ohy_w[:, m], hyb3,
                            float(m), wyb3, OP.is_equal, OP.mult,
                        )
                    ohx_v = ohx.rearrange("p (i t) -> p t i", t=Tc)
                    for g in range(n_grp8):
                        nc.tensor.matmul(
                            ps[:], ohy[:, 128 * g : 128 * (g + 1)],
                            ohx_v[:, 8 * g : 8 * g + 8],
                            start=(mm_i == 0), stop=(mm_i == mm_total - 1),
                        )
                        mm_i += 1

                emit_chunk(0)

                # ---- CIoU elementwise chain (ordered for short live-spans) --
                dx, dy = t_("dx"), t_("dy")
                nc.vector.tensor_tensor(dx, x1, x2, OP.subtract)
                nc.vector.tensor_tensor(dy, y1, y2, OP.subtract)
                adx, ady = t_("adx"), t_("ady")
                nc.scalar.activation(adx, dx, AF.Abs, scale=2.0)
                nc.scalar.activation(ady, dy, AF.Abs, scale=2.0)
                sdx, sdy, rho4 = t_("sdx"), t_("sdy"), t_("rho4")
                nc.scalar.activation(sdx, adx, AF.Square)
                nc.scalar.activation(sdy, ady, AF.Square)
                eng("rho4").tensor_tensor(rho4, sdx, sdy, OP.add)

                dW, dH = t_("dW"), t_("dH")
                nc.vector.tensor_tensor(dW, w1, w2, OP.subtract)
                nc.vector.tensor_tensor(dH, h1, h2, OP.subtract)
                adW, adH = t_("adW"), t_("adH")
                nc.scalar.activation(adW, dW, AF.Abs)
                nc.scalar.activation(adH, dH, AF.Abs)
                mx, my = t_("mx"), t_("my")
                nc.vector.tensor_tensor(mx, adx, adW, OP.max)
                nc.vector.tensor_tensor(my, ady, adH, OP.max)
                W, H = t_("W"), t_("H")
                nc.vector.tensor_tensor(W, w1, w2, OP.add)
                nc.vector.tensor_tensor(H, h1, h2, OP.add)

                iw4, ih4 = t_("iw4"), t_("ih4")
                nc.vector.scalar_tensor_tensor(iw4, mx, -1.0, W, OP.mult, OP.add)
                nc.vector.scalar_tensor_tensor(ih4, my, -1.0, H, OP.mult, OP.add)
                cw2, ch2 = t_("cw2"), t_("ch2")
                nc.vector.tensor_tensor(cw2, W, mx, OP.add)
                nc.vector.tensor_tensor(ch2, H, my, OP.add)
                scw, sch = t_("scw"), t_("sch")
                nc.scalar.activation(scw, cw2, AF.Square)
                nc.scalar.activation(sch, ch2, AF.Square)
                c24 = t_("c24")
                eng("c24").tensor_tensor(c24, scw, sch, OP.add)
                lnc = tf_("lnc")
                r_c = t_("r_c")
                nc.scalar.activation(lnc, c24, AF.Ln, bias=bias_ap(4 * EPS))
                nc.scalar.activation(r_c, lnc, AF.Exp, scale=-1.0)
                term1 = t_("term1")
                nc.vector.tensor_tensor(term1, rho4, r_c, OP.mult)

                ihc, inter4 = t_("ihc"), t_("inter4")
                nc.vector.tensor_scalar(ihc, ih4, 0.0, None, OP.max)
                nc.vector.scalar_tensor_tensor(inter4, iw4, 0.0, ihc, OP.max, OP.mult)
                a2t, a1t, asum = t_("a2t"), t_("a1t"), t_("asum")
                nc.vector.tensor_tensor(a2t, w2, h2, OP.mult)
                nc.vector.tensor_tensor(a1t, w1, h1, OP.mult)
                eng("asum").tensor_tensor(asum, a1t, a2t, OP.add)
                u4 = t_("u4")
                nc.vector.scalar_tensor_tensor(u4, inter4, -0.25, asum, OP.mult, OP.add)
                lnu = tf_("lnu")
                r_u = t_("r_u")
                nc.scalar.activation(lnu, u4, AF.Ln, scale=4.0, bias=bias_ap(4 * EPS))
                nc.scalar.activation(r_u, lnu, AF.Exp, scale=-1.0)
                iou = t_("iou")
                nc.vector.tensor_tensor(iou, inter4, r_u, OP.mult)

                # atan difference
                p21, p12 = t_("p21"), t_("p12")
                nc.vector.tensor_tensor(p21, w2, h1, OP.mult)
                nc.vector.tensor_tensor(p12, w1, h2, OP.mult)
                numt = t_("numt")
                nc.vector.tensor_tensor(numt, p21, p12, OP.subtract)
                anum = t_("anum")
                nc.scalar.activation(anum, numt, AF.Abs)
                phh, pww, dent = t_("phh"), t_("pww"), t_("dent")
                eng("phh").tensor_tensor(phh, h1, h2, OP.mult)
                eng("pww").tensor_tensor(pww, w1, w2, OP.mult)
                eng("dent").tensor_tensor(dent, phh, pww, OP.add)
                mnd, mxd, seld = t_("mnd"), t_("mxd"), t_("seld")
                nc.vector.tensor_tensor(mnd, anum, dent, OP.min)
                nc.vector.tensor_tensor(mxd, anum, dent, OP.max)
                nc.vector.tensor_tensor(seld, anum, dent, OP.is_gt)
                lnm = tf_("lnm")
                rmd = t_("rmd")
                nc.scalar.activation(lnm, mxd, AF.Ln, bias=bias_ap(1e-30))
                nc.scalar.activation(rmd, lnm, AF.Exp, scale=-1.0)
                qr = t_("qr")
                nc.vector.tensor_tensor(qr, mnd, rmd, OP.mult)
                at = t_("at")
                nc.scalar.activation(at, qr, AF.Arctan)
                thd = t_("thd")
                nc.vector.scalar_tensor_tensor(thd, seld, PI / 2, at, OP.mult, OP.subtract)
                vv = t_("vv")
                nc.scalar.activation(vv, thd, AF.Square, scale=2.0 / PI)

                den0 = t_("den0")
                nc.vector.tensor_tensor(den0, vv, iou, OP.subtract)
                lnden = tf_("lnden")
                rden = t_("rden")
                nc.scalar.activation(lnden, den0, AF.Ln, bias=bias_ap(1.0 + EPS))
                nc.scalar.activation(rden, lnden, AF.Exp, scale=-1.0)
                v2 = t_("v2")
                nc.scalar.activation(v2, vv, AF.Square)
                term2, s12, z = t_("term2"), t_("s12"), t_("z")
                eng("term2").tensor_tensor(term2, v2, rden, OP.mult)
                eng("s12").tensor_tensor(s12, term1, term2, OP.add)
                nc.vector.scalar_tensor_tensor(z, iou, -1.0, s12, OP.mult, OP.add)
                om2 = t_("om2")
                nc.scalar.activation(om2, z, AF.Square, bias=bias_ap(1.0))
                lnsw = tf_("lnsw")
                sw = t_("sw")
                nc.scalar.activation(lnsw, a2t, AF.Ln, bias=bias_ap(1e-7))
                nc.scalar.activation(sw, lnsw, AF.Exp, scale=-1.0)
                om3, baset = t_("om3"), t_("baset")
                nc.vector.scalar_tensor_tensor(om3, z, 1.0, om2, OP.add, OP.mult)
                nc.vector.scalar_tensor_tensor(
                    baset, om3, 0.0, sw, OP.add, OP.mult,
                    accum_out=acc_sb[:, n : n + 1],
                )

                for c in range(1, n_chunks):
                    emit_chunk(c)

            # dump the full [128,256] psum; host picks the diagonal cells
            nc.vector.tensor_copy(hist_sb[:], ps[:])
            nc.sync.dma_start(hist_d.ap(), hist_sb[:])
            nc.sync.dma_start(acc_d.ap(), acc_sb[:])

    nc.compile()
    return nc


_CACHE = {}
RUN_KW = {}
LAST_RESULT = None


def _get_program(NB, T, Tc):
    key = (NB, T, Tc)
    if key not in _CACHE:
        _CACHE[key] = build_nc(NB, T=T, Tc=Tc)
    return _CACHE[key]


def _decode_hists(packed_list):
    """Decode per-core psum dumps [128, 256]: real cells sit at
    [8i+s, 32s+j] (i = packed y-row, s = slot, j = x bin); row i packs
    y-bins 2i / 2i+1 at radix 512. Off-diagonal cells are garbage."""
    hist = np.zeros((GRID, GRID), dtype=np.float64)
    ar8 = np.arange(8)
    for p in packed_list:
        P4 = p.reshape(GRID // 2, 8, 8, GRID)       # [i, s_row, s_col, j]
        D = P4[:, ar8, ar8, :]                       # [i, s, j]
        n1 = np.floor(D / 512.0)
        n0 = D - 512.0 * n1
        assert (n0 >= 0).all() and (n0 < 512).all() and (n1 >= 0).all(), "decode overflow"
        hist[0::2, :] += n0.sum(axis=1)
        hist[1::2, :] += n1.sum(axis=1)
    return hist


def kernel(pred_boxes: np.ndarray, target_boxes: np.ndarray) -> np.ndarray:
    N = pred_boxes.shape[0]
    assert N % N_CORES == 0
    n_shard = N // N_CORES
    if N == N_TOTAL:
        NB, T, Tc = NB_CORE, T_MAIN, TC_MAIN
    else:  # generic fallback: tiles of 128x512
        NB = -(-n_shard // 65536) * 65536
        T, Tc = 512, 512
    pad = NB - n_shard
    assert pad >= 0

    pred16 = np.ascontiguousarray(np.asarray(pred_boxes, dtype=np.float16).T)
    targ16 = np.ascontiguousarray(np.asarray(target_boxes, dtype=np.float16).T)
    # f32 coords in (1-2^-12, 1) round up to fp16 1.0 -> floor bin 32; the
    # reference clips bins to 31. Clip the center planes to the largest
    # fp16 below 1.0 so the device's exact-floor binning lands on 31 too.
    np.minimum(targ16[0:2], np.float16(1.0 - 2.0 ** -11), out=targ16[0:2])

    padcol = np.empty((4, pad), np.float16)
    padcol[:] = np.array(PAD_BOX, np.float16)[:, None]

    in_maps = []
    for c in range(N_CORES):
        ps_ = pred16[:, c * n_shard : (c + 1) * n_shard]
        ts_ = targ16[:, c * n_shard : (c + 1) * n_shard]
        if pad:
            ps_ = np.concatenate([ps_, padcol], axis=1)
            ts_ = np.concatenate([ts_, padcol], axis=1)
        in_maps.append({"pred_boxes": np.ascontiguousarray(ps_),
                        "target_boxes": np.ascontiguousarray(ts_)})

    nc = _get_program(NB, T, Tc)
    res = bass_utils.run_bass_kernel_spmd(
        nc, in_maps, core_ids=list(range(N_CORES)), **RUN_KW
    )
    global LAST_RESULT
    LAST_RESULT = res

    base_sum = 0.0
    packed = []
    for r in res.results:
        base_sum += float(r["acc_out"].astype(np.float64).sum())
        packed.append(r["hist_out"].astype(np.float64))
    hist = _decode_hists(packed)
    if pad:
        # pad box center (0.5, 17/32) -> exact floor bin (gy, gx) = (17, 16)
        hist[17, 16] -= pad * N_CORES
    assert hist.sum() == N, (hist.sum(), N)
    mean_base = base_sum / N
    max_h = hist.max()
    result = mean_base * (1.0 + ALPHA * (N / (GRID * GRID)) / max_h)
    return np.float32(result)
